# revision 1
# baseline (speedup 1.0000x reference)
"""Causal self-attention (GPT-style, 12 heads, C=768) on 8 TRN2 NeuronCores.

Sharding: core c -> (batch b = c//2, head-group g = c%2 of 6 heads).
Each core computes qkv projection for its 6 heads, causal attention, and a
partial output projection (its 384 rows of w_proj). Host sums the two
partial projections per batch (row-parallel tensor parallelism) and adds
nothing else (b_proj is folded into the g=0 core's partial).

All matmuls run as float32r (full PE rate at N>=256, ~1e-4 precision).
Layouts chosen so no on-device transposes are needed:
  - x is transposed on host -> xT [C, T]
  - qkv matmul produces qT/kT directly ([head-pair d, T]); V in natural [T, d]
  - scores computed transposed: sT[j, i] = K Q^T via lhsT=kT, rhs=qT
  - softmax denominator via ones-vector matmul (S = sum_j exp)
  - out^T [d, i] = V^T exp accumulated in PSUM, normalized by 1/S broadcast
    (broadcast via K=1 matmul), written as outT [384, T] = proj lhsT.
Head-pairs are packed 2-per-128-partitions: QK uses row-tiled concurrent
matmuls (K=64 at base partition 0/64), AV/S use col-tiled concurrent
matmuls (output at partition base 0/64 resp. 0/32).
"""

import numpy as np

import concourse.bass as bass
import concourse.mybir as mybir
import concourse.tile as tile
from concourse import bacc
from concourse import bass_utils

f32 = mybir.dt.float32
f32r = mybir.dt.float32r
bf16 = mybir.dt.bfloat16
AF = mybir.ActivationFunctionType
ALU = mybir.AluOpType

N_HEAD = 12
N_EMBD = 768
B_FULL = 4
T_FULL = 2048
N_CORES = 8
SCALE = float(N_EMBD) ** -0.5

TRACE = False
LAST_RESULT = None
_NC_CACHE = {}


def build_nc(T=T_FULL, dbg=False):
    """Build the per-core Bass program. All 8 cores run this same program
    on different input data."""
    C = N_EMBD            # 768
    HC = 6                # local heads per core
    NP = 3                # head pairs
    D = 64                # head dim
    KT = C // 128         # 6 k-tiles for the projections
    NIC = T // 512        # i-chunks (512 queries each)
    NJT = T // 128        # j-tiles (128 keys each)

    # Force all ACT ops (softmax Exp + the 1/S Ln/Exp pair) onto the one
    # table set that contains both functions, so the activation-table-load
    # pass emits a single load instead of thrashing between sets. Entry
    # order (and hence act_func_set ids) is preserved; we only hide Exp/Ln
    # from the other sets during this build.
    import concourse.bacc as _bacc_mod
    from concourse.hw_specs import get_activation_tables as _orig_gat

    def _pinned_gat(arch):
        tabs = {k: set(v) for k, v in _orig_gat(arch).items()}
        for name, fns in tabs.items():
            if name != "natural_log_exp_and_others":
                fns.discard(AF.Exp)
                fns.discard(AF.Ln)
        return tabs

    nc = bacc.Bacc("TRN2", target_bir_lowering=False, debug=False)

    xT_d = nc.dram_tensor("xT", [C, T], bf16, kind="ExternalInput")
    wqk_d = nc.dram_tensor("wqk", [C, 768], bf16, kind="ExternalInput")
    wv_d = nc.dram_tensor("wv", [C, 384], bf16, kind="ExternalInput")
    wp_d = nc.dram_tensor("wp", [384, C], bf16, kind="ExternalInput")
    bqk_d = nc.dram_tensor("bqk", [128, 6], f32, kind="ExternalInput")
    bv_d = nc.dram_tensor("bv", [1, 384], bf16, kind="ExternalInput")
    bp_d = nc.dram_tensor("bp", [1, C], bf16, kind="ExternalInput")
    ones_d = nc.dram_tensor("ones", [128, 128], bf16, kind="ExternalInput")
    mask_d = nc.dram_tensor("mask", [128, 2, 128], bf16, kind="ExternalInput")
    y_d = nc.dram_tensor("y", [T, C], f32, kind="ExternalOutput")
    if dbg:
        dbg_qT = nc.dram_tensor("dbg_qT", [128, T], bf16, kind="ExternalOutput")
        dbg_kT = nc.dram_tensor("dbg_kT", [128, T], bf16, kind="ExternalOutput")
        dbg_v = nc.dram_tensor("dbg_v", [128, 390], bf16, kind="ExternalOutput")
        dbg_oT = nc.dram_tensor("dbg_oT", [128, T], bf16, kind="ExternalOutput")
        dbg_ef = nc.dram_tensor("dbg_ef", [128, 1024], bf16, kind="ExternalOutput")
        dbg_ed = nc.dram_tensor("dbg_ed", [128, 512], bf16, kind="ExternalOutput")
        dbg_av = nc.dram_tensor("dbg_av", [128, 512], f32, kind="ExternalOutput")
        dbg_s = nc.dram_tensor("dbg_s", [2, 512], f32, kind="ExternalOutput")
        dbg_rb = nc.dram_tensor("dbg_rb", [128, 512], f32, kind="ExternalOutput")

    with tile.TileContext(nc) as tc:
        with (
            tc.tile_pool(name="const", bufs=1) as constp,
            tc.tile_pool(name="xt", bufs=4) as xtp,
            tc.tile_pool(name="qk", bufs=1) as qkp,
            tc.tile_pool(name="vs", bufs=16) as vsp,
            tc.tile_pool(name="es", bufs=10) as esp,
            tc.tile_pool(name="ot", bufs=1) as otp,
            tc.tile_pool(name="ys", bufs=3) as ysp,
            tc.tile_pool(name="rs", bufs=1) as rsp,
            tc.tile_pool(name="psg", bufs=2, space="PSUM") as psgp,
            tc.tile_pool(name="pav", bufs=2, space="PSUM") as pavp,
        ):
            # ---------------- setup: only what the qkv phase needs -------
            ones = constp.tile([128, 128], bf16, tag="ones")
            nc.sync.dma_start(ones[:], ones_d.ap()[:])
            bv_row = constp.tile([1, 384], bf16, tag="bvr")
            nc.sync.dma_start(bv_row[:], bv_d.ap()[:])
            bqk_t = constp.tile([128, 6], f32, tag="bqk")
            nc.sync.dma_start(bqk_t[:], bqk_d.ap()[:])
            bqk = [bqk_t[:, m:m + 1] for m in range(6)]
            # stripe the startup weight loads across four engine DMA
            # queues so the first qkv matmuls aren't single-queue bound
            wqk_t = constp.tile([128, KT, 768], bf16, tag="wqk")
            wqk_src = wqk_d.ap().rearrange("(k p) c -> p k c", p=128)
            nc.sync.dma_start(wqk_t[:, 0:3, :], wqk_src[:, 0:3, :])
            nc.scalar.dma_start(wqk_t[:, 3:6, :], wqk_src[:, 3:6, :])
            wqk = [wqk_t[:, k, :] for k in range(KT)]
            wv_t = constp.tile([128, KT, 384], bf16, tag="wv")
            wv_src = wv_d.ap().rearrange("(k p) c -> p k c", p=128)
            nc.scalar.dma_start(wv_t[:], wv_src)
            wv = [wv_t[:, k, :] for k in range(KT)]
            bvb = constp.tile([128, 384], f32, tag="bvb")
            ps = psgp.tile([128, 384], f32, tag="sg")
            nc.tensor.matmul(ps[:], ones[0:1, :], bv_row[:], start=True, stop=True)
            nc.vector.tensor_copy(bvb[:], ps[:])

            # ---------------- qkv projection ------------------------------
            # qT/kT: [128 (pair dims), T]; v: per t-tile [128 (t), 384]
            qT = [qkp.tile([128, T], bf16, tag=f"qT{p}", name=f"qT{p}") for p in range(NP)]
            kT = [qkp.tile([128, T], bf16, tag=f"kT{p}", name=f"kT{p}") for p in range(NP)]
            v = [vsp.tile([128, 6, 65], bf16, tag="v", name=f"v{j}") for j in range(NJT)]
            for j in range(NJT):
                nc.vector.memset(v[j][:, :, 64:65], 1.0)

            def emit_qkv_chunk(tci):
                ts512 = slice(512 * tci, 512 * (tci + 1))
                xts_t = xtp.tile([128, KT, 512], bf16, tag="xt")
                nc.gpsimd.dma_start(
                    xts_t[:],
                    xT_d.ap().rearrange("(k p) t -> p k t", p=128)[:, :, ts512])
                xts = [xts_t[:, k, :] for k in range(KT)]
                # qT / kT  (m 0..2 -> q pairs, 3..5 -> k pairs)
                for m in range(6):
                    ps = psgp.tile([128, 512], f32, tag="sg")
                    for k in range(KT):
                        nc.tensor.matmul(ps[:], wqk[k][:, 128 * m:128 * (m + 1)],
                                         xts[k],
                                         start=(k == 0), stop=(k == KT - 1))
                    dest = qT[m] if m < 3 else kT[m - 3]
                    nc.vector.tensor_scalar_add(dest[:, ts512], ps[:], bqk[m])
                # v natural layout
                for tsub in range(4):
                    jt = 4 * tci + tsub
                    ps = psgp.tile([128, 384], f32, tag="sg")
                    for k in range(KT):
                        nc.tensor.matmul(
                            ps[:],
                            xts[k][:, 128 * tsub:128 * (tsub + 1)],
                            wv[k],
                            start=(k == 0), stop=(k == KT - 1))
                    nc.vector.tensor_tensor(
                        v[jt][:, :, 0:64],
                        ps[:].rearrange("p (h d) -> p h d", h=6),
                        bvb[:].rearrange("p (h d) -> p h d", h=6),
                        op=ALU.add)

            # ---- late constants (mask, proj weights/bias); emitted after
            #      the first qkv chunk so startup DMAs stay minimal ---------
            def emit_late_consts():
                msk = constp.tile([128, 2, 128], bf16, tag="msk")
                nc.sync.dma_start(msk[:], mask_d.ap()[:])
                wp_t = constp.tile([128, NP, 768], bf16, tag="wp")
                nc.sync.dma_start(
                    wp_t[:], wp_d.ap().rearrange("(m p) c -> p m c", p=128))
                wp = [wp_t[:, m, :] for m in range(NP)]
                bp_row = constp.tile([1, 768], bf16, tag="bpr")
                nc.sync.dma_start(bp_row[:], bp_d.ap()[:])
                bpb = constp.tile([128, 768], f32, tag="bpb")
                ps = psgp.tile([128, 768], f32, tag="sg")
                for lo, hi in [(0, 512), (512, 768)]:
                    nc.tensor.matmul(ps[:, lo:hi], ones[0:1, :],
                                     bp_row[:, lo:hi], start=True, stop=True)
                nc.vector.tensor_copy(bpb[:], ps[:])
                return msk, wp, bpb

            # ---------------- attention + projection ----------------------
            outT = [otp.tile([128, T], bf16, tag=f"outT{p}", name=f"outT{p}") for p in range(NP)]

            def emit_attn_chunk(ic):
                isl = slice(512 * ic, 512 * (ic + 1))
                njt = 4 * ic + 4          # j-tiles for this i-chunk (incl diag 4)
                ngr = njt // 2            # score groups of 2 j-tiles
                for p in range(NP):
                    pairs = [(0, slice(0, 64)), (1, slice(64, 128))]
                    # Pipeline per group of 2 j-tiles: scores (row-tiled head
                    # pair) -> exp (+causal mask on the 2 diagonal-straddling
                    # groups) -> AV accumulation (M=65: V plus a ones column,
                    # so row 64 of the psum accumulates the softmax sum S).
                    av = {h: pavp.tile([65, 512], f32, tag="av", name=f"av{h}",
                       bufs=2)
                          for h, _ in pairs}
                    # ---- full score groups (2 j-tiles x 512 queries each) --
                    for gi in range(2 * ic):
                        ets = {}
                        sgs = {}
                        for h, dsl in pairs:
                            sgs[h] = psgp.tile([128, 2, 512], f32, tag="sg",
                                               name=f"sg{h}")
                        # interleave head A/B: consecutive matmuls hit
                        # different PE row groups (K=64 at base 0 vs 64) and
                        # run concurrently in separate sub-arrays
                        for j2 in range(2):
                            jt = 2 * gi + j2
                            for h, dsl in pairs:
                                nc.tensor.matmul(
                                    sgs[h][:, j2, :],
                                    kT[p][dsl, 128 * jt:128 * (jt + 1)],
                                    qT[p][dsl, isl],
                                    start=True, stop=True,
                                    tile_position=(64 * h, 0))
                        for h, dsl in pairs:
                            et = esp.tile([128, 2, 512], bf16, tag=f"e{h}")
                            nc.scalar.activation(et[:], sgs[h][:], AF.Exp,
                                                 scale=SCALE)
                            ets[h] = et
                        for h, dsl in pairs:
                            hl = 2 * p + h
                            for j2 in range(2):
                                jt = 2 * gi + j2
                                nc.tensor.matmul(
                                    av[h][:, :], v[jt][:, hl, :],
                                    ets[h][:, j2, :],
                                    start=(jt == 0), stop=False)

                    # ---- diagonal stripe: decreasing-width blocks ----------
                    # j-tile 4ic+d covers queries i_local in [128d, 512); only
                    # its leading 128 columns need the causal tril mask.
                    # Packed bank-aligned into two psum tiles: [512|384] and
                    # [256|128].
                    dspec = [
                        # (grp, col0, width, i0, start, stop)  (per sg bank)
                        (0, 0, 512, 0, True, True),
                        (0, 512, 384, 128, True, True),
                        (1, 0, 256, 256, True, False),
                        (1, 256, 128, 384, False, True),
                    ]
                    sgd = {}
                    for h, dsl in pairs:
                        sgd[h] = [psgp.tile([128, 896], f32, tag="sg",
                                            name=f"sgd{h}0"),
                                  pavp.tile([128, 384], f32, tag="sm",
                                            name=f"sgd{h}1", bufs=2)]
                    eds = {}
                    for d4, (grp, c0, w, i0, st, sp) in enumerate(dspec[:2]):
                        jt = 4 * ic + d4
                        for h, dsl in pairs:
                            nc.tensor.matmul(
                                sgd[h][grp][:, c0:c0 + w],
                                kT[p][dsl, 128 * jt:128 * (jt + 1)],
                                qT[p][dsl, 512 * ic + i0:512 * ic + i0 + w],
                                start=st, stop=sp,
                                tile_position=(64 * h, 0))
                    for h, dsl in pairs:
                        # exp the 2-bank stripe right away so its sg slots
                        # free before the next section's score matmuls
                        e1 = esp.tile([128, 896], bf16, tag=f"e{h}")
                        nc.scalar.activation(e1[:], sgd[h][0][:], AF.Exp,
                                             scale=SCALE)
                        v1 = e1[:].rearrange("p (a b) -> p a b", b=128)[:, 0::4, :]
                        nc.vector.tensor_tensor(v1, v1, msk[:], op=ALU.mult)
                        eds[h] = [e1, None]
                    for d4, (grp, c0, w, i0, st, sp) in enumerate(dspec[2:]):
                        jt = 4 * ic + 2 + d4
                        for h, dsl in pairs:
                            nc.tensor.matmul(
                                sgd[h][grp][:, c0:c0 + w],
                                kT[p][dsl, 128 * jt:128 * (jt + 1)],
                                qT[p][dsl, 512 * ic + i0:512 * ic + i0 + w],
                                start=st, stop=sp,
                                tile_position=(64 * h, 0))
                    for h, dsl in pairs:
                        e2 = esp.tile([128, 384], bf16, tag=f"e{h}")
                        nc.scalar.activation(e2[:], sgd[h][1][:], AF.Exp,
                                             scale=SCALE)
                        v2 = e2[:].rearrange("p (a b) -> p a b", b=128)[:, 0::2, :]
                        nc.vector.tensor_tensor(v2, v2, msk[:], op=ALU.mult)
                        eds[h][1] = e2
                    for d4, (grp, c0, w, i0, st, sp) in enumerate(dspec):
                        jt = 4 * ic + d4
                        for h, dsl in pairs:
                            hl = 2 * p + h
                            nc.tensor.matmul(
                                av[h][:, i0:512], v[jt][:, hl, :],
                                eds[h][grp][:, c0:c0 + w],
                                start=(ic == 0 and d4 == 0), stop=(d4 == 3))
                    if dbg and p == 0 and ic == min(NIC - 1, 1):
                        tdbg = ysp.tile([128, 512], f32, tag="y", name="tdbg")
                        nc.vector.tensor_copy(tdbg[0:64, :], av[0][0:64, :])
                        nc.vector.tensor_copy(tdbg[64:128, :], av[1][0:64, :])
                        nc.sync.dma_start(dbg_av.ap()[:], tdbg[:])

                    # -- normalize: outT_h = av_h[0:64] * (1/S_h) where
                    #    S_h = av_h[64] (ones-column sum); 1/S broadcast to 64
                    #    partitions via a K=1 matmul
                    # 1/S = exp(-ln S) on ScalarE (same ACT table set as
                    # the softmax exp); both heads pipelined stage-by-stage
                    rrs = {}
                    rfs = {}
                    for h, dsl in pairs:
                        rf = rsp.tile([1, 512], f32, tag="rf", bufs=2)
                        nc.scalar.activation(rf[:], av[h][64:65, :], AF.Ln)
                        rfs[h] = rf
                    for h, dsl in pairs:
                        rr = rsp.tile([1, 512], bf16, tag="rr", bufs=2)
                        nc.scalar.activation(rr[:], rfs[h][:], AF.Exp, scale=-1.0)
                        rrs[h] = rr
                    rbps = {}
                    for h, dsl in pairs:
                        rbp = pavp.tile([64, 512], f32, tag="sm", bufs=2)
                        nc.tensor.matmul(rbp[:], ones[0:1, 0:64], rrs[h][:],
                                         start=True, stop=True)
                        rbps[h] = rbp
                    for h, dsl in pairs:
                        rbs = rsp.tile([64, 512], f32, tag="rbs", bufs=2)
                        nc.vector.tensor_copy(rbs[:], rbps[h][:])
                        if dbg and p == 0 and ic == min(NIC - 1, 1):
                            nc.sync.dma_start(
                                dbg_rb.ap()[64 * h:64 * h + 64, :], rbs[:])
                        nc.vector.tensor_tensor(outT[p][dsl, isl],
                                                av[h][0:64, :], rbs[:],
                                                op=ALU.mult)

                if dbg and ic == NIC - 1:
                    nc.sync.dma_start(dbg_oT.ap()[:], outT[0][:])
                # -- output projection for this i-chunk
                for tsub in range(4):
                    t0 = 512 * ic + 128 * tsub
                    ysb = ysp.tile([128, 768], f32, tag="y")
                    for n in range(2):
                        nsl = slice(384 * n, 384 * (n + 1))
                        yp = pavp.tile([128, 384], f32, tag="sm", bufs=2)
                        for mp in range(NP):
                            nc.tensor.matmul(
                                yp[:], outT[mp][:, t0:t0 + 128],
                                wp[mp][:, nsl],
                                start=(mp == 0), stop=(mp == NP - 1))
                        nc.vector.tensor_tensor(ysb[:, nsl], yp[:], bpb[:, nsl],
                                                op=ALU.add)
                        nc.sync.dma_start(y_d.ap()[t0:t0 + 128, nsl],
                                          ysb[:, nsl])

            # ---------------- pipelined emission --------------------------
            # qkv chunk ic feeds attention chunk ic immediately, so ScalarE
            # exp work starts ~60us earlier and overlaps the remaining
            # projection matmuls instead of pacing the kernel at the end.
            emit_qkv_chunk(0)
            msk, wp, bpb = emit_late_consts()
            emit_attn_chunk(0)
            for ic in range(1, NIC):
                emit_qkv_chunk(ic)
                emit_attn_chunk(ic)

    _bacc_mod.get_activation_tables = _pinned_gat
    try:
        nc.compile()
    finally:
        _bacc_mod.get_activation_tables = _orig_gat
    return nc


def make_in_maps(x, w_attn, b_attn, w_proj, b_proj, T=T_FULL):
    import ml_dtypes
    bf = ml_dtypes.bfloat16
    x = np.asarray(x, np.float32)
    w_attn = np.asarray(w_attn, np.float32)
    b_attn = np.asarray(b_attn, np.float32)
    w_proj = np.asarray(w_proj, np.float32)
    b_proj = np.asarray(b_proj, np.float32)
    B = x.shape[0]

    ones = np.ones((128, 128), bf)
    # tril mask for the leading 128-column diagonal sub-block of each
    # stripe matmul (two identical copies to keep the AP stride regular)
    mask = np.broadcast_to(
        (np.arange(128)[:, None, None] <= np.arange(128)[None, None, :]),
        (128, 2, 128)).astype(np.float32)

    in_maps = []
    for c in range(N_CORES):
        b, g = (c // 2) % B, c % 2
        q0, k0, v0 = 384 * g, 768 + 384 * g, 1536 + 384 * g
        wqk = np.concatenate(
            [w_attn[:, q0:q0 + 384], w_attn[:, k0:k0 + 384]], axis=1)
        bqk = np.concatenate(
            [b_attn[q0:q0 + 384], b_attn[k0:k0 + 384]])
        in_maps.append({
            "xT": np.ascontiguousarray(x[b].T).astype(bf),
            "wqk": np.ascontiguousarray(wqk).astype(bf),
            "wv": np.ascontiguousarray(w_attn[:, v0:v0 + 384]).astype(bf),
            "wp": np.ascontiguousarray(w_proj[384 * g:384 * (g + 1), :]).astype(bf),
            "bqk": np.ascontiguousarray(bqk.reshape(6, 128).T),
            "bv": np.ascontiguousarray(b_attn[v0:v0 + 384].reshape(1, 384)).astype(bf),
            "bp": np.ascontiguousarray(
                (b_proj if g == 0 else np.zeros_like(b_proj)).reshape(1, -1)).astype(bf),
            "ones": ones,
            "mask": np.ascontiguousarray(mask).astype(bf),
        })
    return in_maps


def kernel(x, w_attn, b_attn, w_proj, b_proj):
    global LAST_RESULT
    if "nc" not in _NC_CACHE:
        _NC_CACHE["nc"] = build_nc(T_FULL)
    nc = _NC_CACHE["nc"]
    in_maps = make_in_maps(x, w_attn, b_attn, w_proj, b_proj)
    res = bass_utils.run_bass_kernel_spmd(
        nc, in_maps, core_ids=list(range(N_CORES)), trace=TRACE)
    LAST_RESULT = res
    B, T, C = np.asarray(x).shape
    y = np.empty((B, T, C), np.float32)
    for b in range(B):
        y[b] = res.results[2 * b]["y"] + res.results[2 * b + 1]["y"]
    return y



# revision 3
# speedup vs baseline: 1.2521x; 1.2521x over previous
"""Causal self-attention (GPT-style, 12 heads, C=768) on 8 TRN2 NeuronCores.

Sharding: core c -> (batch b = c//2, head-group g = c%2 of 6 heads).
Each core computes qkv projection for its 6 heads, causal attention, and a
partial output projection (its 384 rows of w_proj). Host sums the two
partial projections per batch (row-parallel tensor parallelism); b_proj is
folded into the g=0 core's partial.

v2 pipeline design (HAM-warm scheduling):
  - score psum groups are per-j-tile [128, 2(heads), 512] tiles (2 banks),
    ring of 2 -> depth-2 software pipeline: scores(g+1) issue while exp(g)
    runs on ScalarE, AV(g) follows. One exp op covers both heads.
  - AV accumulates into one [65, 2, 512] psum (M=65: V plus a ones column
    so row 64 accumulates the softmax denominator S).
  - softmax normalization is deferred: av rows 0..63 are copied to outT
    unnormalized; S rows are gathered (partitions 0/32/64 of an SBUF
    table) and 1/S = exp(-ln S) is computed once per chunk in one batched
    Ln + Exp pair. Per (pair, head): K=1 broadcast matmul of 1/S then one
    in-place DVE multiply on outT.
  - independent matmul work (next chunk's qkv projection, previous chunk's
    normalize broadcasts + output projection) is interleaved between
    attention groups as PE filler so the tensor engine never idles long
    enough for the HAM clock gate to re-throttle it to 1.2 GHz.
  - single shared 2-slot PSUM scratch ring serves qkv/proj/broadcast;
    8 PSUM banks total: 4 (scores) + 2 (AV) + 2 (scratch).
"""

import numpy as np

import concourse.bass as bass
import concourse.mybir as mybir
import concourse.tile as tile
from concourse import bacc
from concourse import bass_utils

f32 = mybir.dt.float32
bf16 = mybir.dt.bfloat16
AF = mybir.ActivationFunctionType
ALU = mybir.AluOpType

N_HEAD = 12
N_EMBD = 768
B_FULL = 4
T_FULL = 2048
N_CORES = 8
SCALE = float(N_EMBD) ** -0.5

TRACE = False
LAST_RESULT = None
_NC_CACHE = {}


def build_nc(T=T_FULL, dbg=False):
    C = N_EMBD            # 768
    NP = 3                # head pairs (6 local heads)
    KT = C // 128         # 6 k-tiles for the projections
    NIC = T // 512        # i-chunks (512 queries each)
    NJT = T // 128        # j-tiles (128 keys each)

    # Pin Exp/Ln to the one activation-table set containing both, so the
    # table-load pass emits a single load.
    import concourse.bacc as _bacc_mod
    from concourse.hw_specs import get_activation_tables as _orig_gat

    def _pinned_gat(arch):
        tabs = {k: set(v) for k, v in _orig_gat(arch).items()}
        for name, fns in tabs.items():
            if name != "natural_log_exp_and_others":
                fns.discard(AF.Exp)
                fns.discard(AF.Ln)
        return tabs

    nc = bacc.Bacc("TRN2", target_bir_lowering=False, debug=False)

    xT_d = nc.dram_tensor("xT", [C, T], bf16, kind="ExternalInput")
    wqk_d = nc.dram_tensor("wqk", [C, 768], bf16, kind="ExternalInput")
    wv_d = nc.dram_tensor("wv", [C, 384], bf16, kind="ExternalInput")
    wp_d = nc.dram_tensor("wp", [384, C], bf16, kind="ExternalInput")
    bqk_d = nc.dram_tensor("bqk", [128, 6], f32, kind="ExternalInput")
    bv_d = nc.dram_tensor("bv", [1, 384], bf16, kind="ExternalInput")
    bp_d = nc.dram_tensor("bp", [1, C], bf16, kind="ExternalInput")
    ones_d = nc.dram_tensor("ones", [128, 128], bf16, kind="ExternalInput")
    mask_d = nc.dram_tensor("mask", [128, 2, 128], bf16, kind="ExternalInput")
    y_d = nc.dram_tensor("y", [T, C], f32, kind="ExternalOutput")

    with tile.TileContext(nc) as tc:
        with (
            tc.tile_pool(name="const", bufs=1) as constp,
            tc.tile_pool(name="xt", bufs=1) as xtp,
            tc.tile_pool(name="qk", bufs=1) as qkp,
            tc.tile_pool(name="vs", bufs=16) as vsp,
            tc.tile_pool(name="es", bufs=6) as esp,
            tc.tile_pool(name="ot", bufs=1) as otp,
            tc.tile_pool(name="sn", bufs=1) as snp,
            tc.tile_pool(name="ys", bufs=4) as ysp,
            tc.tile_pool(name="psg", bufs=2, space="PSUM") as psgp,
            tc.tile_pool(name="pav", bufs=1, space="PSUM") as pavp,
            tc.tile_pool(name="psc", bufs=2, space="PSUM") as pscp,
        ):
            # ---------------- startup constants + prefetch ----------------
            ones = constp.tile([128, 128], bf16, tag="ones")
            nc.sync.dma_start(ones[:], ones_d.ap()[:])
            bv_row = constp.tile([1, 384], bf16, tag="bvr")
            nc.sync.dma_start(bv_row[:], bv_d.ap()[:])
            bqk_t = constp.tile([128, 6], f32, tag="bqk")
            nc.sync.dma_start(bqk_t[:], bqk_d.ap()[:])
            bqk = [bqk_t[:, m:m + 1] for m in range(6)]

            wqk_t = constp.tile([128, KT, 768], bf16, tag="wqk")
            wqk_src = wqk_d.ap().rearrange("(k p) c -> p k c", p=128)
            nc.sync.dma_start(wqk_t[:, 0:2, :], wqk_src[:, 0:2, :])
            nc.scalar.dma_start(wqk_t[:, 2:4, :], wqk_src[:, 2:4, :])
            nc.gpsimd.dma_start(wqk_t[:, 4:6, :], wqk_src[:, 4:6, :])
            wqk = [wqk_t[:, k, :] for k in range(KT)]
            wv_t = constp.tile([128, KT, 384], bf16, tag="wv")
            wv_src = wv_d.ap().rearrange("(k p) c -> p k c", p=128)
            nc.scalar.dma_start(wv_t[:], wv_src)
            wv = [wv_t[:, k, :] for k in range(KT)]

            # all x chunks prefetched up front (striped across DGE queues)
            xts_tiles = []
            for ic in range(NIC):
                xt_ic = xtp.tile([128, KT, 512], bf16, tag=f"xt{ic}",
                                 name=f"xt{ic}")
                eng = (nc.gpsimd, nc.sync, nc.scalar, nc.gpsimd)[ic % 4]
                eng.dma_start(
                    xt_ic[:],
                    xT_d.ap().rearrange("(k p) t -> p k t", p=128)
                    [:, :, 512 * ic:512 * (ic + 1)])
                xts_tiles.append(xt_ic)

            # softmax-denominator table: rows at partitions 32p hold, per
            # chunk, [2 heads x 512] sums; memset so Ln of unused rows is
            # well-defined.
            S_all = snp.tile([65, NIC, 2, 512], f32, tag="sall")
            nc.vector.memset(S_all[:], 1.0)
            rr_all = snp.tile([65, NIC, 2, 512], bf16, tag="rrall")

            # v tiles: [keys 128, local head, 64 v-dims + ones column]
            v = [vsp.tile([128, 6, 65], bf16, tag="v", name=f"v{j}")
                 for j in range(NJT)]
            for j in range(NJT):
                nc.vector.memset(v[j][:, :, 64:65], 1.0)

            # bvb: bias row broadcast to 128 partitions via K=1 matmul
            bvb = constp.tile([128, 384], f32, tag="bvb")
            ps0 = pscp.tile([128, 512], f32, tag="sc", name="ps_bvb")
            nc.tensor.matmul(ps0[:, 0:384], ones[0:1, :], bv_row[:],
                             start=True, stop=True)
            nc.vector.tensor_copy(bvb[:], ps0[:, 0:384])
            bvb_r = bvb[:].rearrange("p (h d) -> p h d", h=6)

            # ---------------- qkv projection units ------------------------
            qT = [qkp.tile([128, T], bf16, tag=f"qT{p}", name=f"qT{p}")
                  for p in range(NP)]
            kT = [qkp.tile([128, T], bf16, tag=f"kT{p}", name=f"kT{p}")
                  for p in range(NP)]

            def qk_unit(tci, m):
                def emit():
                    ps = pscp.tile([128, 512], f32, tag="sc",
                                   name=f"psqk{tci}_{m}")
                    xts = xts_tiles[tci]
                    for k in range(KT):
                        nc.tensor.matmul(ps[:],
                                         wqk[k][:, 128 * m:128 * (m + 1)],
                                         xts[:, k, :],
                                         start=(k == 0), stop=(k == KT - 1))
                    dest = qT[m] if m < 3 else kT[m - 3]
                    nc.vector.tensor_scalar_add(
                        dest[:, 512 * tci:512 * (tci + 1)], ps[:], bqk[m])
                return emit

            def v_unit(tci, tsub):
                def emit():
                    ps = pscp.tile([128, 512], f32, tag="sc",
                                   name=f"psv{tci}_{tsub}")
                    xts = xts_tiles[tci]
                    jt = 4 * tci + tsub
                    for k in range(KT):
                        nc.tensor.matmul(
                            ps[:, 0:384],
                            xts[:, k, 128 * tsub:128 * (tsub + 1)],
                            wv[k], start=(k == 0), stop=(k == KT - 1))
                    nc.vector.tensor_tensor(
                        v[jt][:, :, 0:64],
                        ps[:, 0:384].rearrange("p (h d) -> p h d", h=6),
                        bvb_r, op=ALU.add)
                return emit

            def qkv_units(tci):
                us = [qk_unit(tci, m) for m in range(6)]
                us += [v_unit(tci, tsub) for tsub in range(4)]
                return us

            # ---------------- late consts (proj weights, mask) ------------
            outT = [otp.tile([128, T], bf16, tag=f"outT{p}", name=f"outT{p}")
                    for p in range(NP)]
            late = {}

            def emit_late_consts():
                msk = constp.tile([128, 2, 128], bf16, tag="msk")
                nc.sync.dma_start(msk[:], mask_d.ap()[:])
                wp_t = constp.tile([128, NP, 768], bf16, tag="wp")
                nc.scalar.dma_start(
                    wp_t[:], wp_d.ap().rearrange("(m p) c -> p m c", p=128))
                bp_row = constp.tile([1, 768], bf16, tag="bpr")
                nc.sync.dma_start(bp_row[:], bp_d.ap()[:])
                bpb = constp.tile([128, 768], f32, tag="bpb")
                for lo, hi in [(0, 512), (512, 768)]:
                    ps = pscp.tile([128, 512], f32, tag="sc", name="ps_bpb")
                    nc.tensor.matmul(ps[:, 0:hi - lo], ones[0:1, :],
                                     bp_row[:, lo:hi], start=True, stop=True)
                    nc.vector.tensor_copy(bpb[:, lo:hi], ps[:, 0:hi - lo])
                late["msk"] = msk
                late["wp"] = [wp_t[:, m, :] for m in range(NP)]
                late["bpb"] = bpb

            # ---------------- normalize + projection units ----------------
            def norm_unit(ic, p, h):
                def emit():
                    dsl = slice(64 * h, 64 * (h + 1))
                    isl = slice(512 * ic, 512 * (ic + 1))
                    sl = pscp.tile([128, 512], f32, tag="sc",
                                   name=f"rbp{ic}_{p}_{h}")
                    rbp = sl[0:64, :]
                    nc.tensor.matmul(rbp,
                                     ones[32 * p:32 * p + 1, 0:64],
                                     rr_all[32 * p:32 * p + 1, ic, h, :],
                                     start=True, stop=True)
                    nc.vector.tensor_tensor(outT[p][dsl, isl],
                                            outT[p][dsl, isl], rbp,
                                            op=ALU.mult)
                return emit

            def proj_unit(ic, tsub):
                def emit():
                    wp, bpb = late["wp"], late["bpb"]
                    t0 = 512 * ic + 128 * tsub
                    ysb = ysp.tile([128, 768], f32, tag="y",
                                   name=f"y{ic}_{tsub}")
                    for n in range(2):
                        nsl = slice(384 * n, 384 * (n + 1))
                        sl = pscp.tile([128, 512], f32, tag="sc",
                                       name=f"yp{ic}_{tsub}_{n}")
                        yp = sl[:, 0:384]
                        for mp in range(NP):
                            nc.tensor.matmul(
                                yp, outT[mp][:, t0:t0 + 128], wp[mp][:, nsl],
                                start=(mp == 0), stop=(mp == NP - 1))
                        nc.vector.tensor_tensor(ysb[:, nsl], yp,
                                                bpb[:, nsl], op=ALU.add)
                        nc.sync.dma_start(y_d.ap()[t0:t0 + 128, nsl],
                                          ysb[:, nsl])
                return emit

            def norm_proj_units(ic):
                us = [norm_unit(ic, p, h) for p in range(NP) for h in (0, 1)]
                us += [proj_unit(ic, tsub) for tsub in range(4)]
                return us

            # ---------------- attention chunk -----------------------------
            def emit_attn_chunk(ic, filler):
                isl = slice(512 * ic, 512 * (ic + 1))
                msk = late["msk"]
                fill_credit = [0.0]
                n_groups = NP * (4 * ic + 3)
                quota = len(filler) / max(n_groups, 1)
                fi = [0]

                def drain_filler():
                    fill_credit[0] += quota
                    while fi[0] < len(filler) and fill_credit[0] >= 1.0:
                        filler[fi[0]]()
                        fi[0] += 1
                        fill_credit[0] -= 1.0

                pairs = [(0, slice(0, 64)), (1, slice(64, 128))]

                def emit_scores(p, g):
                    kind = g[0]
                    if kind == "full":
                        jt = g[1]
                        sg = psgp.tile([128, 2, 512], f32, tag="sg",
                                       name=f"sg{ic}_{p}_{jt}")
                        for h, dsl in pairs:
                            nc.tensor.matmul(
                                sg[:, h, :],
                                kT[p][dsl, 128 * jt:128 * (jt + 1)],
                                qT[p][dsl, isl], start=True, stop=True)
                        return (sg, 512)
                    if kind == "d0":
                        jt = 4 * ic
                        sg = psgp.tile([128, 2, 512], f32, tag="sg",
                                       name=f"sgd0_{ic}_{p}")
                        for h, dsl in pairs:
                            nc.tensor.matmul(
                                sg[:, h, :],
                                kT[p][dsl, 128 * jt:128 * (jt + 1)],
                                qT[p][dsl, isl], start=True, stop=True)
                        return (sg, 512)
                    if kind == "d1":
                        jt = 4 * ic + 1
                        sg = psgp.tile([128, 2, 512], f32, tag="sg",
                                       name=f"sgd1_{ic}_{p}")
                        for h, dsl in pairs:
                            nc.tensor.matmul(
                                sg[:, h, 0:384],
                                kT[p][dsl, 128 * jt:128 * (jt + 1)],
                                qT[p][dsl, 512 * ic + 128:512 * ic + 512],
                                start=True, stop=True)
                        return (sg, 384)
                    # d23: two decreasing blocks packed per head
                    sg = psgp.tile([128, 2, 512], f32, tag="sg",
                                   name=f"sgd23_{ic}_{p}")
                    for h, dsl in pairs:
                        nc.tensor.matmul(
                            sg[:, h, 0:256],
                            kT[p][dsl, 128 * (4 * ic + 2):128 * (4 * ic + 3)],
                            qT[p][dsl, 512 * ic + 256:512 * ic + 512],
                            start=True, stop=False)
                        nc.tensor.matmul(
                            sg[:, h, 256:384],
                            kT[p][dsl, 128 * (4 * ic + 3):128 * (4 * ic + 4)],
                            qT[p][dsl, 512 * ic + 384:512 * ic + 512],
                            start=False, stop=True)
                    return (sg, 384)

                def emit_exp_av(p, g, sg_w, avt, first):
                    kind = g[0]
                    sg, w = sg_w
                    et = esp.tile([128, 2, w], bf16, tag="et",
                                  name=f"et{ic}_{p}_{kind}")
                    nc.scalar.activation(et[:], sg[:, :, 0:w], AF.Exp,
                                         scale=SCALE)
                    if kind == "d0" or kind == "d1":
                        nc.vector.tensor_tensor(et[:, :, 0:128],
                                                et[:, :, 0:128], msk[:],
                                                op=ALU.mult)
                    elif kind == "d23":
                        nc.vector.tensor_tensor(et[:, :, 0:128],
                                                et[:, :, 0:128], msk[:],
                                                op=ALU.mult)
                        nc.vector.tensor_tensor(et[:, :, 256:384],
                                                et[:, :, 256:384], msk[:],
                                                op=ALU.mult)
                    for h, dsl in pairs:
                        hl = 2 * p + h
                        if kind == "full":
                            jt = g[1]
                            nc.tensor.matmul(avt[:, h, :], v[jt][:, hl, :],
                                             et[:, h, :],
                                             start=first, stop=False)
                        elif kind == "d0":
                            nc.tensor.matmul(avt[:, h, :],
                                             v[4 * ic][:, hl, :],
                                             et[:, h, :],
                                             start=first, stop=False)
                        elif kind == "d1":
                            nc.tensor.matmul(avt[:, h, 128:512],
                                             v[4 * ic + 1][:, hl, :],
                                             et[:, h, :],
                                             start=False, stop=False)
                        else:
                            nc.tensor.matmul(avt[:, h, 256:512],
                                             v[4 * ic + 2][:, hl, :],
                                             et[:, h, 0:256],
                                             start=False, stop=False)
                            nc.tensor.matmul(avt[:, h, 384:512],
                                             v[4 * ic + 3][:, hl, :],
                                             et[:, h, 256:384],
                                             start=False, stop=True)

                for p in range(NP):
                    avt = pavp.tile([65, 2, 512], f32, tag="av",
                                    name=f"av{ic}_{p}")
                    glist = [("full", jt) for jt in range(4 * ic)]
                    glist += [("d0",), ("d1",), ("d23",)]
                    sg_w = emit_scores(p, glist[0])
                    for i, g in enumerate(glist):
                        cur = sg_w
                        if i + 1 < len(glist):
                            sg_w = emit_scores(p, glist[i + 1])
                        drain_filler()
                        emit_exp_av(p, g, cur, avt, first=(i == 0))
                    # evacuate: unnormalized outT + S rows
                    for h, dsl in pairs:
                        nc.vector.tensor_copy(outT[p][dsl, isl],
                                              avt[0:64, h, :])
                    nc.vector.tensor_copy(S_all[32 * p:32 * p + 1, ic],
                                          avt[64:65, :, :])
                # leftover filler
                while fi[0] < len(filler):
                    filler[fi[0]]()
                    fi[0] += 1
                # batched 1/S for this chunk: rr = exp(-ln S)
                lnS = snp.tile([65, 2, 512], f32, tag="lnS", bufs=2,
                               name=f"lnS{ic}")
                nc.scalar.activation(lnS[:], S_all[:, ic], AF.Ln)
                nc.scalar.activation(rr_all[:, ic], lnS[:], AF.Exp,
                                     scale=-1.0)

            # ---------------- main schedule -------------------------------
            for u in qkv_units(0):
                u()
            emit_late_consts()
            emit_attn_chunk(0, qkv_units(1))
            emit_attn_chunk(1, qkv_units(2) + norm_proj_units(0))
            emit_attn_chunk(2, qkv_units(3) + norm_proj_units(1))
            emit_attn_chunk(3, norm_proj_units(2))
            for u in norm_proj_units(3):
                u()

    _bacc_mod.get_activation_tables = _pinned_gat
    try:
        nc.compile()
    finally:
        _bacc_mod.get_activation_tables = _orig_gat
    return nc


def make_in_maps(x, w_attn, b_attn, w_proj, b_proj, T=T_FULL):
    import ml_dtypes
    bf = ml_dtypes.bfloat16
    x = np.asarray(x, np.float32)
    w_attn = np.asarray(w_attn, np.float32)
    b_attn = np.asarray(b_attn, np.float32)
    w_proj = np.asarray(w_proj, np.float32)
    b_proj = np.asarray(b_proj, np.float32)
    B = x.shape[0]

    ones = np.ones((128, 128), bf)
    # tril mask for the leading 128-column diagonal sub-block of each
    # stripe matmul (two identical copies, one per head)
    mask = np.broadcast_to(
        (np.arange(128)[:, None, None] <= np.arange(128)[None, None, :]),
        (128, 2, 128)).astype(np.float32)

    in_maps = []
    for c in range(N_CORES):
        b, g = (c // 2) % B, c % 2
        q0, k0, v0 = 384 * g, 768 + 384 * g, 1536 + 384 * g
        wqk = np.concatenate(
            [w_attn[:, q0:q0 + 384], w_attn[:, k0:k0 + 384]], axis=1)
        bqk = np.concatenate(
            [b_attn[q0:q0 + 384], b_attn[k0:k0 + 384]])
        in_maps.append({
            "xT": np.ascontiguousarray(x[b].T).astype(bf),
            "wqk": np.ascontiguousarray(wqk).astype(bf),
            "wv": np.ascontiguousarray(w_attn[:, v0:v0 + 384]).astype(bf),
            "wp": np.ascontiguousarray(w_proj[384 * g:384 * (g + 1), :]).astype(bf),
            "bqk": np.ascontiguousarray(bqk.reshape(6, 128).T),
            "bv": np.ascontiguousarray(b_attn[v0:v0 + 384].reshape(1, 384)).astype(bf),
            "bp": np.ascontiguousarray(
                (b_proj if g == 0 else np.zeros_like(b_proj)).reshape(1, -1)).astype(bf),
            "ones": ones,
            "mask": np.ascontiguousarray(mask).astype(bf),
        })
    return in_maps


def kernel(x, w_attn, b_attn, w_proj, b_proj):
    global LAST_RESULT
    if "nc" not in _NC_CACHE:
        _NC_CACHE["nc"] = build_nc(T_FULL)
    nc = _NC_CACHE["nc"]
    in_maps = make_in_maps(x, w_attn, b_attn, w_proj, b_proj)
    res = bass_utils.run_bass_kernel_spmd(
        nc, in_maps, core_ids=list(range(N_CORES)), trace=TRACE)
    LAST_RESULT = res
    B, T, C = np.asarray(x).shape
    y = np.empty((B, T, C), np.float32)
    for b in range(B):
        y[b] = res.results[2 * b]["y"] + res.results[2 * b + 1]["y"]
    return y
